# revision 6
# baseline (speedup 1.0000x reference)
"""IsoVelo kNN cosine-similarity loss on 8 Trainium2 NeuronCores.

Strategy: data-parallel over the 100k cells in blocks of 12544 per core
(128 partitions x 98 cells; the last core is zero-padded). Host sends
only unique, compact data: per core its shard of the fp16 state table
[12544,17], its shard of the fp16 velocity v = predict - state
[12544,17], and neighbor indices packed as uint16 low halves plus one
int32 of 30 high bits per cell (17-bit indices). On device the state
shards are AllGathered into a full fp16 table (rank blocks of 12544
rows make global row == cell id), indices are unpacked with DVE integer
ops, then neighbor rows are fetched with indirect DMA gathers (34B
rows). Per-pair cosine math runs on DVE/ACT in fp16/fp32; per-core
partial sums are reduced with a 1-wide PE matmul and summed on host.

Wall-clock wins over the naive approach are host-side: ~15MB shipped
per call instead of ~80MB (no 8x-replicated fp32 table), a jitted
executable cached in-process so repeat calls skip re-tracing and the
8x-repeated remote output fetch, and the jax persistent compilation
cache so a fresh process compiles once.
"""

import os
import tempfile

import numpy as np
import jax

_cache_dir = os.path.join(tempfile.gettempdir(), "jax_comp_cache")
try:
    jax.config.update("jax_compilation_cache_dir", _cache_dir)
    jax.config.update("jax_persistent_cache_min_compile_time_secs", 0)
    jax.config.update("jax_persistent_cache_min_entry_size_bytes", 0)
except Exception:
    pass

import concourse.bass as bass
import concourse.bacc as bacc
import concourse.mybir as mybir
from concourse.bass import AP, IndirectOffsetOnAxis
from concourse.tile import TileContext
from concourse import bass_utils

F32 = mybir.dt.float32
F16 = mybir.dt.float16
I32 = mybir.dt.int32
U16 = mybir.dt.uint16

N_CELLS = 100000
N_ISO = 16
D = N_ISO + 1          # 17
K = 30                 # neighbors per cell (indices[:, 1:31])
N_CORES = 8
PER_PART = 98          # cells per partition
SHARD = 128 * PER_PART          # 12544 cells per core (zero-padded globally)
PAD_TOTAL = N_CORES * SHARD     # 100352
T = 7                  # cells per partition per chunk
NCH = PER_PART // T    # 14 chunks per core
PK = T * K             # 210 pairs per partition per chunk
PY = PK * D            # 3570 gathered elements per partition per chunk

_CACHED = {}


def _fv(ap, dims):
    """View a tile AP with custom free dims (list of [step, count] in
    elements), keeping its partition entry."""
    return AP(ap.tensor, ap.offset, [ap.ap[0]] + [list(d) for d in dims])


def _ov(ap, off, dims):
    return AP(ap.tensor, ap.offset + off, [ap.ap[0]] + [list(d) for d in dims])


def _pview(dram, elems_per_part):
    """View a row-major DRAM param as [128, elems_per_part]: partition p
    reads elems_per_part contiguous elements at offset p*elems_per_part."""
    ap = dram[:]
    return AP(ap.tensor, 0, [[elems_per_part, 128], [1, elems_per_part]])


def _build_bass():
    nc = bacc.Bacc(num_devices=N_CORES)
    tabsh = nc.declare_dram_parameter("tabsh", [SHARD, D], F16, isOutput=False)
    vsh = nc.declare_dram_parameter("vsh", [SHARD, D], F16, isOutput=False)
    lo = nc.declare_dram_parameter("lo", [SHARD, K], U16, isOutput=False)
    hi = nc.declare_dram_parameter("hi", [SHARD, 1], I32, isOutput=False)
    out = nc.declare_dram_parameter("out", [1, 1], F32, isOutput=True)

    tabfull = nc.dram_tensor("tabfull", [PAD_TOTAL, D], F16, addr_space="Shared")

    with TileContext(nc) as tc:
        with (
            tc.tile_pool(name="dram", bufs=1, space="DRAM") as dp,
            tc.tile_pool(name="const", bufs=1) as cp,
            tc.tile_pool(name="io", bufs=3) as iop,
            tc.tile_pool(name="big", bufs=2) as bp,
            tc.tile_pool(name="small", bufs=2) as sp,
            tc.tile_pool(name="psum", bufs=1, space="PSUM") as pp,
        ):
            # AllGather the fp16 state-table shard into the full table.
            myin = dp.tile([SHARD, D], F16)
            nc.gpsimd.dma_start(myin[:], tabsh[:])
            nc.gpsimd.collective_compute(
                "AllGather",
                mybir.AluOpType.bypass,
                replica_groups=[list(range(N_CORES))],
                ins=[myin.opt()],
                outs=[tabfull[:].opt()],
            )

            acc = cp.tile([128, 1], F32)
            ones = cp.tile([128, 1], F32)
            nc.vector.memset(acc[:], 0.0)
            nc.vector.memset(ones[:], 1.0)

            # Resident shard data, partition-major by construction:
            # partition p owns cells [p*98, (p+1)*98).
            tab_s = cp.tile([128, PER_PART * D], F16)
            v_s = cp.tile([128, PER_PART * D], F16)
            lo_s = cp.tile([128, PER_PART * K], U16)
            hi_s = cp.tile([128, PER_PART], I32)
            nc.sync.dma_start(out=tab_s[:], in_=_pview(tabsh, PER_PART * D))
            nc.sync.dma_start(out=v_s[:], in_=_pview(vsh, PER_PART * D))
            nc.sync.dma_start(out=lo_s[:], in_=_pview(lo, PER_PART * K))
            nc.sync.dma_start(out=hi_s[:], in_=_pview(hi, PER_PART))

            # Unpack 17-bit indices: idx = lo + (((hi >> k) & 1) << 16).
            kv = cp.tile([128, K], I32)
            nc.gpsimd.iota(kv[:], pattern=[[1, K]], base=0, channel_multiplier=0)
            sh_t = cp.tile([128, PER_PART * K], I32)
            nc.vector.tensor_tensor(
                out=_fv(sh_t[:], [[K, PER_PART], [1, K]]),
                in0=_fv(hi_s[:], [[1, PER_PART], [0, K]]),
                in1=_fv(kv[:], [[0, PER_PART], [1, K]]),
                op=mybir.AluOpType.logical_shift_right,
            )
            b_t = cp.tile([128, PER_PART * K], I32)
            nc.vector.tensor_scalar(
                out=b_t[:], in0=sh_t[:], scalar1=1, scalar2=16,
                op0=mybir.AluOpType.bitwise_and,
                op1=mybir.AluOpType.logical_shift_left,
            )
            lo32 = cp.tile([128, PER_PART * K], I32)
            nc.vector.tensor_copy(out=lo32[:], in_=lo_s[:])
            idx_s = cp.tile([128, PER_PART * K], I32)
            nc.vector.tensor_add(out=idx_s[:], in0=lo32[:], in1=b_t[:])

            # |v|^2 per cell, all 98 cells at once.
            vsq = cp.tile([128, PER_PART * D], F32)
            nc.scalar.square(out=vsq[:], in_=v_s[:])
            vn2 = cp.tile([128, PER_PART], F32)
            nc.vector.tensor_reduce(
                out=vn2[:], in_=_fv(vsq[:], [[D, PER_PART], [1, D]]),
                axis=mybir.AxisListType.X, op=mybir.AluOpType.add,
            )

            for ch in range(NCH):
                el_off = ch * T * D            # element offset of chunk in tab_s/v_s
                idxsl = idx_s[:, ch * PK:(ch + 1) * PK]

                Y = iop.tile([128, PY], F16, tag="Y")
                nc.gpsimd.indirect_dma_start(
                    out=Y[:],
                    out_offset=None,
                    in_=tabfull[:],
                    in_offset=IndirectOffsetOnAxis(ap=idxsl, axis=0),
                )

                # neighbor displacement vn = Y - x (x broadcast over K)
                vn = bp.tile([128, PY], F16, tag="vn")
                Y4 = _fv(Y[:], [[K * D, T], [D, K], [1, D]])
                xb = _ov(tab_s[:], el_off, [[D, T], [0, K], [1, D]])
                vn4 = _fv(vn[:], [[K * D, T], [D, K], [1, D]])
                nc.vector.tensor_tensor(
                    out=vn4, in0=Y4, in1=xb, op=mybir.AluOpType.subtract
                )

                # dots = sum_d vn * v (v broadcast over K)
                tt = bp.tile([128, PY], F16, tag="scratch")
                vb = _ov(v_s[:], el_off, [[D, T], [0, K], [1, D]])
                tt4 = _fv(tt[:], [[K * D, T], [D, K], [1, D]])
                nc.vector.tensor_tensor(out=tt4, in0=vn4, in1=vb, op=mybir.AluOpType.mult)
                dots = sp.tile([128, PK], F32, tag="dots")
                nc.vector.tensor_reduce(
                    out=dots[:], in_=tt4,
                    axis=mybir.AxisListType.X, op=mybir.AluOpType.add,
                )

                # d2 = |vn|^2 (square on ACT to offload DVE)
                t2 = bp.tile([128, PY], F32, tag="sq")
                nc.scalar.square(out=t2[:], in_=vn[:])
                d2 = sp.tile([128, PK], F32, tag="d2")
                nc.vector.tensor_reduce(
                    out=d2[:], in_=_fv(t2[:], [[K * D, T], [D, K], [1, D]]),
                    axis=mybir.AxisListType.X, op=mybir.AluOpType.add,
                )

                # denom^2 = d2 * |v|^2, clamped away from zero.
                # Exact-duplicate neighbors (j == i) give vn == 0 bit-exactly,
                # so dots == 0 and the clamped ratio is 0, matching the
                # reference's "denom==0 -> cos=dots" guard.
                d2v = sp.tile([128, PK], F32, tag="d2v")
                vn2b = _ov(vn2[:], ch * T, [[1, T], [0, K]])
                nc.vector.tensor_tensor(
                    out=_fv(d2v[:], [[K, T], [1, K]]),
                    in0=_fv(d2[:], [[K, T], [1, K]]),
                    in1=vn2b, op=mybir.AluOpType.mult,
                )
                nc.vector.tensor_scalar_max(d2v[:], d2v[:], 1e-30)

                q = sp.tile([128, PK], F32, tag="q")
                nc.scalar.sqrt(out=q[:], in_=d2v[:])
                r = sp.tile([128, PK], F32, tag="r")
                nc.vector.reciprocal(out=r[:], in_=q[:])
                s = sp.tile([128, PK], F32, tag="s")
                nc.vector.tensor_mul(out=s[:], in0=dots[:], in1=r[:])

                # max over neighbors, then accumulate per partition
                m = sp.tile([128, T], F32, tag="m")
                nc.vector.tensor_reduce(
                    out=m[:], in_=_fv(s[:], [[K, T], [1, K]]),
                    axis=mybir.AxisListType.X, op=mybir.AluOpType.max,
                )
                msum = sp.tile([128, 1], F32, tag="msum")
                nc.vector.tensor_reduce(
                    out=msum[:], in_=m[:],
                    axis=mybir.AxisListType.X, op=mybir.AluOpType.add,
                )
                nc.vector.tensor_add(out=acc[:], in0=acc[:], in1=msum[:])

            ps = pp.tile([1, 1], F32)
            nc.tensor.matmul(out=ps[:], lhsT=acc[:], rhs=ones[:], start=True, stop=True)
            sres = cp.tile([1, 1], F32)
            nc.vector.tensor_copy(out=sres[:], in_=ps[:])
            nc.sync.dma_start(out=out[:], in_=sres[:])

    nc.compile()
    return nc


def _get_nc():
    if "nc" not in _CACHED:
        _CACHED["nc"] = _build_bass()
    return _CACHED["nc"]


class _Runner:
    """In-process cached SPMD runner: the jitted shard_map executable is
    built once, so repeat calls skip re-tracing/compile-cache lookups and
    fetch each output exactly once. Mirrors bass2jax.run_bass_via_pjrt."""

    def __init__(self, nc, n_cores):
        from concourse import bass2jax

        bass2jax.install_neuronx_cc_hook()
        assert nc.dbg_addr is None
        partition_name = (
            nc.partition_id_tensor.name if nc.partition_id_tensor else None
        )

        in_names, out_names, out_avals = [], [], []
        for alloc in nc.m.functions[0].allocations:
            if not isinstance(alloc, mybir.MemoryLocationSet):
                continue
            name = alloc.memorylocations[0].name
            if alloc.kind == "ExternalInput":
                if name != partition_name:
                    in_names.append(name)
            elif alloc.kind == "ExternalOutput":
                out_names.append(name)
                out_avals.append(jax.core.ShapedArray(
                    tuple(alloc.tensor_shape), mybir.dt.np(alloc.dtype)))
        self.in_names = list(in_names)
        self.n_cores = n_cores
        self._out_info = [(tuple(a.shape), a.dtype) for a in out_avals]
        n_in, n_out = len(in_names), len(out_names)
        all_in_names = tuple(
            in_names + out_names + ([partition_name] if partition_name else [])
        )

        def _body(*args):
            operands = list(args)
            if partition_name is not None:
                operands.append(bass2jax.partition_id_tensor())
            outs = bass2jax._bass_exec_p.bind(
                *operands,
                out_avals=tuple(out_avals),
                in_names=all_in_names,
                out_names=tuple(out_names),
                lowering_input_output_aliases=(),
                sim_require_finite=True,
                sim_require_nnan=True,
                nc=nc,
            )
            return tuple(outs)

        from jax.experimental.shard_map import shard_map
        from jax.sharding import Mesh, PartitionSpec

        devices = jax.devices()[:n_cores]
        assert len(devices) == n_cores
        mesh = Mesh(np.asarray(devices), ("core",))
        self._fn = jax.jit(
            shard_map(
                _body, mesh=mesh,
                in_specs=(PartitionSpec("core"),) * (n_in + n_out),
                out_specs=(PartitionSpec("core"),) * n_out,
                check_rep=False,
            ),
            donate_argnums=tuple(range(n_in, n_in + n_out)),
            keep_unused=True,
        )

    def __call__(self, arrays_by_name):
        ins = [arrays_by_name[n] for n in self.in_names]
        zeros = [np.zeros((self.n_cores * s[0], *s[1:]), d)
                 for s, d in self._out_info]
        outs = self._fn(*ins, *zeros)
        return [np.asarray(o) for o in outs]


def _get_runner(nc):
    if "runner" not in _CACHED:
        _CACHED["runner"] = _Runner(nc, N_CORES)
    return _CACHED["runner"]


def _prepare_global(unsplice, splices, unsplice_predict, splice_predicts, indices):
    u = np.asarray(unsplice, dtype=np.float32).reshape(N_CELLS)
    s = np.asarray(splices, dtype=np.float32).reshape(N_CELLS, N_ISO)
    up = np.asarray(unsplice_predict, dtype=np.float32).reshape(N_CELLS)
    sp_ = np.asarray(splice_predicts, dtype=np.float32).reshape(N_CELLS, N_ISO)
    out = {}

    def _mk_tab():
        tabsh = np.zeros((PAD_TOTAL, D), dtype=np.float16)
        tabsh[:N_CELLS, 0] = u
        tabsh[:N_CELLS, 1:] = s
        out["tabsh"] = tabsh

    def _mk_v():
        vsh = np.zeros((PAD_TOTAL, D), dtype=np.float16)
        vsh[:N_CELLS, 0] = up - u
        vsh[:N_CELLS, 1:] = sp_ - s
        out["vsh"] = vsh

    def _mk_idx():
        idx = np.asarray(indices).reshape(N_CELLS, K + 1)[:, 1:]
        if idx.dtype != np.int32:
            idx = idx.astype(np.int32)
        lo = np.zeros((PAD_TOTAL, K), dtype=np.uint16)
        lo[:N_CELLS] = idx.astype(np.uint16)
        hi = np.zeros((PAD_TOTAL, 1), dtype=np.int32)
        # bit k of hi = (idx[:,k] >= 2^16); packbits LSB-first -> 4B -> i32
        hi[:N_CELLS] = np.packbits(
            idx >= 65536, axis=1, bitorder="little").view(np.int32)
        out["lo"] = lo
        out["hi"] = hi

    from concurrent.futures import ThreadPoolExecutor
    with ThreadPoolExecutor(3) as pool:
        futs = [pool.submit(f) for f in (_mk_idx, _mk_tab, _mk_v)]
        for f in futs:
            f.result()
    return out


def _split_per_core(arrays):
    in_maps = []
    for c in range(N_CORES):
        sl = slice(c * SHARD, (c + 1) * SHARD)
        in_maps.append({k: v[sl] for k, v in arrays.items()})
    return in_maps


def kernel(unsplice, splices, unsplice_predict, splice_predicts, indices,
           _trace=False):
    nc = _get_nc()
    arrays = _prepare_global(
        unsplice, splices, unsplice_predict, splice_predicts, indices
    )
    if _trace or _CACHED.get("fallback"):
        res = bass_utils.run_bass_kernel_spmd(
            nc, _split_per_core(arrays), list(range(N_CORES)), trace=_trace
        )
        S = sum(float(res.results[i]["out"][0, 0]) for i in range(N_CORES))
        loss = np.float32(1.0 - S / N_CELLS)
        if _trace:
            return loss, res
        return loss
    try:
        outs = _get_runner(nc)(arrays)
        S = float(outs[0].sum())
    except Exception:
        _CACHED["fallback"] = True
        res = bass_utils.run_bass_kernel_spmd(
            nc, _split_per_core(arrays), list(range(N_CORES))
        )
        S = sum(float(res.results[i]["out"][0, 0]) for i in range(N_CORES))
    return np.float32(1.0 - S / N_CELLS)


# revision 7
# speedup vs baseline: 1.0717x; 1.0717x over previous
"""IsoVelo kNN cosine-similarity loss on 8 Trainium2 NeuronCores.

Strategy: data-parallel over the 100k cells in blocks of 12544 per core
(128 partitions x 98 cells; the last core is zero-padded). Host sends
one packed uint16 blob per core, [12544, 66]: per 132-byte row the fp16
state table entry (17), the fp16 velocity v = predict - state (17), the
neighbor indices' uint16 low halves (30), and two uint16 words holding
the 30 index high bits (17-bit indices). On device each core carves its
fp16 state rows out of the blob, AllGathers them into a full fp16 table
(rank blocks of 12544 rows make global row == cell id), unpacks indices
with DVE integer ops, then fetches neighbor rows with indirect DMA
gathers (34B rows). Per-pair cosine math runs on DVE/ACT in fp16/fp32;
per-core partial sums are reduced with a 1-wide PE matmul and summed on
the host.

Wall-clock wins over the naive approach are host-side: ~13MB shipped
per call in one array instead of ~80MB in several (no 8x-replicated
fp32 table), a jitted executable cached in-process so repeat calls skip
re-tracing and repeated remote output fetches, and the jax persistent
compilation cache so a fresh process can reuse the compiled NEFF.
"""

import os
import tempfile
from concurrent.futures import ThreadPoolExecutor

import numpy as np
import jax

_cache_dir = os.path.join(tempfile.gettempdir(), "jax_comp_cache")
try:
    jax.config.update("jax_compilation_cache_dir", _cache_dir)
    jax.config.update("jax_persistent_cache_min_compile_time_secs", 0)
    jax.config.update("jax_persistent_cache_min_entry_size_bytes", 0)
except Exception:
    pass

import concourse.bass as bass
import concourse.bacc as bacc
import concourse.mybir as mybir
from concourse.bass import AP, IndirectOffsetOnAxis
from concourse.tile import TileContext
from concourse import bass_utils

F32 = mybir.dt.float32
F16 = mybir.dt.float16
I32 = mybir.dt.int32
U16 = mybir.dt.uint16

N_CELLS = 100000
N_ISO = 16
D = N_ISO + 1          # 17
K = 30                 # neighbors per cell (indices[:, 1:31])
N_CORES = 8
PER_PART = 98          # cells per partition
SHARD = 128 * PER_PART          # 12544 cells per core (zero-padded globally)
PAD_TOTAL = N_CORES * SHARD     # 100352
T = 7                  # cells per partition per chunk
NCH = PER_PART // T    # 14 chunks per core
PK = T * K             # 210 pairs per partition per chunk
PY = PK * D            # 3570 gathered elements per partition per chunk
ROW = 2 * D + K + 2    # 66 uint16 per packed row
OFF_TAB = 0            # f16 state, elements [0, 17)
OFF_V = D              # f16 velocity, elements [17, 34)
OFF_LO = 2 * D         # u16 index low halves, [34, 64)
OFF_H0 = 2 * D + K     # u16 high bits of idx k=0..15
OFF_H1 = OFF_H0 + 1    # u16 high bits of idx k=16..29

_CACHED = {}


def _fv(ap, dims):
    """View a tile AP with custom free dims (list of [step, count] in
    elements), keeping its partition entry."""
    return AP(ap.tensor, ap.offset, [ap.ap[0]] + [list(d) for d in dims])


def _ov(ap, off, dims):
    return AP(ap.tensor, ap.offset + off, [ap.ap[0]] + [list(d) for d in dims])


def _build_bass():
    nc = bacc.Bacc(num_devices=N_CORES)
    blob = nc.declare_dram_parameter("blob", [SHARD, ROW], U16, isOutput=False)
    out = nc.declare_dram_parameter("out", [1, 1], F32, isOutput=True)

    tabfull = nc.dram_tensor("tabfull", [PAD_TOTAL, D], F16, addr_space="Shared")
    blob16 = blob.bitcast(F16)

    with TileContext(nc) as tc:
        with (
            tc.tile_pool(name="dram", bufs=1, space="DRAM") as dp,
            tc.tile_pool(name="const", bufs=1) as cp,
            tc.tile_pool(name="io", bufs=3) as iop,
            tc.tile_pool(name="big", bufs=2) as bp,
            tc.tile_pool(name="small", bufs=2) as sp,
            tc.tile_pool(name="psum", bufs=1, space="PSUM") as pp,
        ):
            # Extract this core's fp16 state rows and AllGather into the
            # full table.
            myin = dp.tile([SHARD, D], F16)
            nc.gpsimd.dma_start(
                myin[:], AP(blob16, OFF_TAB, [[ROW, SHARD], [1, D]])
            )
            nc.gpsimd.collective_compute(
                "AllGather",
                mybir.AluOpType.bypass,
                replica_groups=[list(range(N_CORES))],
                ins=[myin.opt()],
                outs=[tabfull[:].opt()],
            )

            acc = cp.tile([128, 1], F32)
            ones = cp.tile([128, 1], F32)
            nc.vector.memset(acc[:], 0.0)
            nc.vector.memset(ones[:], 1.0)

            # Resident packed shard, partition-major by construction:
            # partition p owns cells [p*98, (p+1)*98).
            blob_s = cp.tile([128, PER_PART * ROW], U16)
            nc.sync.dma_start(
                out=blob_s[:],
                in_=AP(blob[:].tensor, 0,
                       [[PER_PART * ROW, 128], [1, PER_PART * ROW]]),
            )
            bs16 = blob_s[:].bitcast(F16)

            # Unpack 17-bit indices: idx = lo + (((h >> k) & 1) << 16),
            # where h's bits 0..15 come from word h0 and 16..29 from h1.
            kv = cp.tile([128, K], I32)
            nc.gpsimd.iota(kv[:], pattern=[[1, K]], base=0, channel_multiplier=0)
            h0 = cp.tile([128, PER_PART], I32)
            h1 = cp.tile([128, PER_PART], I32)
            nc.vector.tensor_copy(
                out=h0[:], in_=_ov(blob_s[:], OFF_H0, [[ROW, PER_PART]]))
            nc.vector.tensor_copy(
                out=h1[:], in_=_ov(blob_s[:], OFF_H1, [[ROW, PER_PART]]))
            sh_t = cp.tile([128, PER_PART * K], I32)
            nc.vector.tensor_tensor(
                out=_fv(sh_t[:], [[K, PER_PART], [1, 16]]),
                in0=_fv(h0[:], [[1, PER_PART], [0, 16]]),
                in1=_fv(kv[:], [[0, PER_PART], [1, 16]]),
                op=mybir.AluOpType.logical_shift_right,
            )
            nc.vector.tensor_tensor(
                out=_ov(sh_t[:], 16, [[K, PER_PART], [1, K - 16]]),
                in0=_fv(h1[:], [[1, PER_PART], [0, K - 16]]),
                in1=_fv(kv[:], [[0, PER_PART], [1, K - 16]]),
                op=mybir.AluOpType.logical_shift_right,
            )
            nc.vector.tensor_scalar(
                out=sh_t[:], in0=sh_t[:], scalar1=1, scalar2=16,
                op0=mybir.AluOpType.bitwise_and,
                op1=mybir.AluOpType.logical_shift_left,
            )
            idx_s = cp.tile([128, PER_PART * K], I32)
            nc.vector.tensor_copy(
                out=idx_s[:],
                in_=_ov(blob_s[:], OFF_LO, [[ROW, PER_PART], [1, K]]))
            nc.vector.tensor_add(out=idx_s[:], in0=idx_s[:], in1=sh_t[:])

            # |v|^2 per cell, all 98 cells at once.
            vsq = cp.tile([128, PER_PART * D], F32)
            nc.scalar.square(
                out=vsq[:], in_=_ov(bs16, OFF_V, [[ROW, PER_PART], [1, D]]))
            vn2 = cp.tile([128, PER_PART], F32)
            nc.vector.tensor_reduce(
                out=vn2[:], in_=_fv(vsq[:], [[D, PER_PART], [1, D]]),
                axis=mybir.AxisListType.X, op=mybir.AluOpType.add,
            )

            for ch in range(NCH):
                row_off = ch * T * ROW         # f16-element offset of chunk
                idxsl = idx_s[:, ch * PK:(ch + 1) * PK]

                Y = iop.tile([128, PY], F16, tag="Y")
                nc.gpsimd.indirect_dma_start(
                    out=Y[:],
                    out_offset=None,
                    in_=tabfull[:],
                    in_offset=IndirectOffsetOnAxis(ap=idxsl, axis=0),
                )

                # neighbor displacement vn = Y - x (x broadcast over K)
                vn = bp.tile([128, PY], F16, tag="vn")
                Y4 = _fv(Y[:], [[K * D, T], [D, K], [1, D]])
                xb = _ov(bs16, row_off + OFF_TAB, [[ROW, T], [0, K], [1, D]])
                vn4 = _fv(vn[:], [[K * D, T], [D, K], [1, D]])
                nc.vector.tensor_tensor(
                    out=vn4, in0=Y4, in1=xb, op=mybir.AluOpType.subtract
                )

                # dots = sum_d vn * v (v broadcast over K)
                tt = bp.tile([128, PY], F16, tag="scratch")
                vb = _ov(bs16, row_off + OFF_V, [[ROW, T], [0, K], [1, D]])
                tt4 = _fv(tt[:], [[K * D, T], [D, K], [1, D]])
                nc.vector.tensor_tensor(out=tt4, in0=vn4, in1=vb, op=mybir.AluOpType.mult)
                dots = sp.tile([128, PK], F32, tag="dots")
                nc.vector.tensor_reduce(
                    out=dots[:], in_=tt4,
                    axis=mybir.AxisListType.X, op=mybir.AluOpType.add,
                )

                # d2 = |vn|^2 (square on ACT to offload DVE)
                t2 = bp.tile([128, PY], F32, tag="sq")
                nc.scalar.square(out=t2[:], in_=vn[:])
                d2 = sp.tile([128, PK], F32, tag="d2")
                nc.vector.tensor_reduce(
                    out=d2[:], in_=_fv(t2[:], [[K * D, T], [D, K], [1, D]]),
                    axis=mybir.AxisListType.X, op=mybir.AluOpType.add,
                )

                # denom^2 = d2 * |v|^2, clamped away from zero.
                # Exact-duplicate neighbors (j == i) give vn == 0 bit-exactly,
                # so dots == 0 and the clamped ratio is 0, matching the
                # reference's "denom==0 -> cos=dots" guard.
                d2v = sp.tile([128, PK], F32, tag="d2v")
                vn2b = _ov(vn2[:], ch * T, [[1, T], [0, K]])
                nc.vector.tensor_tensor(
                    out=_fv(d2v[:], [[K, T], [1, K]]),
                    in0=_fv(d2[:], [[K, T], [1, K]]),
                    in1=vn2b, op=mybir.AluOpType.mult,
                )
                nc.vector.tensor_scalar_max(d2v[:], d2v[:], 1e-30)

                q = sp.tile([128, PK], F32, tag="q")
                nc.scalar.sqrt(out=q[:], in_=d2v[:])
                r = sp.tile([128, PK], F32, tag="r")
                nc.vector.reciprocal(out=r[:], in_=q[:])
                s = sp.tile([128, PK], F32, tag="s")
                nc.vector.tensor_mul(out=s[:], in0=dots[:], in1=r[:])

                # max over neighbors, then accumulate per partition
                m = sp.tile([128, T], F32, tag="m")
                nc.vector.tensor_reduce(
                    out=m[:], in_=_fv(s[:], [[K, T], [1, K]]),
                    axis=mybir.AxisListType.X, op=mybir.AluOpType.max,
                )
                msum = sp.tile([128, 1], F32, tag="msum")
                nc.vector.tensor_reduce(
                    out=msum[:], in_=m[:],
                    axis=mybir.AxisListType.X, op=mybir.AluOpType.add,
                )
                nc.vector.tensor_add(out=acc[:], in0=acc[:], in1=msum[:])

            ps = pp.tile([1, 1], F32)
            nc.tensor.matmul(out=ps[:], lhsT=acc[:], rhs=ones[:], start=True, stop=True)
            sres = cp.tile([1, 1], F32)
            nc.vector.tensor_copy(out=sres[:], in_=ps[:])
            nc.sync.dma_start(out=out[:], in_=sres[:])

    nc.compile()
    return nc


def _get_nc():
    if "nc" not in _CACHED:
        _CACHED["nc"] = _build_bass()
    return _CACHED["nc"]


class _Runner:
    """In-process cached SPMD runner: the jitted shard_map executable is
    built once, so repeat calls skip re-tracing/compile-cache lookups and
    fetch each output exactly once. Mirrors bass2jax.run_bass_via_pjrt."""

    def __init__(self, nc, n_cores):
        from concourse import bass2jax

        bass2jax.install_neuronx_cc_hook()
        assert nc.dbg_addr is None
        partition_name = (
            nc.partition_id_tensor.name if nc.partition_id_tensor else None
        )

        in_names, out_names, out_avals = [], [], []
        for alloc in nc.m.functions[0].allocations:
            if not isinstance(alloc, mybir.MemoryLocationSet):
                continue
            name = alloc.memorylocations[0].name
            if alloc.kind == "ExternalInput":
                if name != partition_name:
                    in_names.append(name)
            elif alloc.kind == "ExternalOutput":
                out_names.append(name)
                out_avals.append(jax.core.ShapedArray(
                    tuple(alloc.tensor_shape), mybir.dt.np(alloc.dtype)))
        self.in_names = list(in_names)
        self.n_cores = n_cores
        self._out_info = [(tuple(a.shape), a.dtype) for a in out_avals]
        n_in, n_out = len(in_names), len(out_names)
        all_in_names = tuple(
            in_names + out_names + ([partition_name] if partition_name else [])
        )

        def _body(*args):
            operands = list(args)
            if partition_name is not None:
                operands.append(bass2jax.partition_id_tensor())
            outs = bass2jax._bass_exec_p.bind(
                *operands,
                out_avals=tuple(out_avals),
                in_names=all_in_names,
                out_names=tuple(out_names),
                lowering_input_output_aliases=(),
                sim_require_finite=True,
                sim_require_nnan=True,
                nc=nc,
            )
            return tuple(outs)

        from jax.experimental.shard_map import shard_map
        from jax.sharding import Mesh, PartitionSpec

        devices = jax.devices()[:n_cores]
        assert len(devices) == n_cores
        mesh = Mesh(np.asarray(devices), ("core",))
        self._fn = jax.jit(
            shard_map(
                _body, mesh=mesh,
                in_specs=(PartitionSpec("core"),) * (n_in + n_out),
                out_specs=(PartitionSpec("core"),) * n_out,
                check_rep=False,
            ),
            donate_argnums=tuple(range(n_in, n_in + n_out)),
            keep_unused=True,
        )

    def __call__(self, arrays_by_name):
        ins = [arrays_by_name[n] for n in self.in_names]
        zeros = [np.zeros((self.n_cores * s[0], *s[1:]), d)
                 for s, d in self._out_info]
        outs = self._fn(*ins, *zeros)
        return [np.asarray(o) for o in outs]


def _get_runner(nc):
    if "runner" not in _CACHED:
        _CACHED["runner"] = _Runner(nc, N_CORES)
    return _CACHED["runner"]


def _get_pool():
    if "pool" not in _CACHED:
        _CACHED["pool"] = ThreadPoolExecutor(3)
    return _CACHED["pool"]


def _prepare_global(unsplice, splices, unsplice_predict, splice_predicts, indices):
    u = np.asarray(unsplice, dtype=np.float32).reshape(N_CELLS)
    s = np.asarray(splices, dtype=np.float32).reshape(N_CELLS, N_ISO)
    up = np.asarray(unsplice_predict, dtype=np.float32).reshape(N_CELLS)
    sp_ = np.asarray(splice_predicts, dtype=np.float32).reshape(N_CELLS, N_ISO)

    buf = np.zeros((PAD_TOTAL, ROW), dtype=np.uint16)
    f16 = buf.view(np.float16)

    def _mk_tab():
        f16[:N_CELLS, OFF_TAB] = u
        f16[:N_CELLS, OFF_TAB + 1:OFF_TAB + D] = s

    def _mk_v():
        f16[:N_CELLS, OFF_V] = up - u
        f16[:N_CELLS, OFF_V + 1:OFF_V + D] = sp_ - s

    def _mk_idx():
        idx = np.asarray(indices).reshape(N_CELLS, K + 1)[:, 1:]
        if idx.dtype != np.int32:
            idx = idx.astype(np.int32)
        buf[:N_CELLS, OFF_LO:OFF_LO + K] = idx.astype(np.uint16)
        # bit k = (idx[:,k] >= 2^16); packbits LSB-first -> 4 bytes -> 2 u16
        buf[:N_CELLS, OFF_H0:OFF_H0 + 2] = np.packbits(
            idx >= 65536, axis=1, bitorder="little").view(np.uint16)

    futs = [_get_pool().submit(f) for f in (_mk_idx, _mk_tab, _mk_v)]
    for f in futs:
        f.result()
    return {"blob": buf}


def _split_per_core(arrays):
    in_maps = []
    for c in range(N_CORES):
        sl = slice(c * SHARD, (c + 1) * SHARD)
        in_maps.append({k: v[sl] for k, v in arrays.items()})
    return in_maps


def kernel(unsplice, splices, unsplice_predict, splice_predicts, indices,
           _trace=False):
    nc = _get_nc()
    arrays = _prepare_global(
        unsplice, splices, unsplice_predict, splice_predicts, indices
    )
    if _trace or _CACHED.get("fallback"):
        res = bass_utils.run_bass_kernel_spmd(
            nc, _split_per_core(arrays), list(range(N_CORES)), trace=_trace
        )
        S = sum(float(res.results[i]["out"][0, 0]) for i in range(N_CORES))
        loss = np.float32(1.0 - S / N_CELLS)
        if _trace:
            return loss, res
        return loss
    try:
        outs = _get_runner(nc)(arrays)
        S = float(outs[0].sum())
    except Exception:
        _CACHED["fallback"] = True
        res = bass_utils.run_bass_kernel_spmd(
            nc, _split_per_core(arrays), list(range(N_CORES))
        )
        S = sum(float(res.results[i]["out"][0, 0]) for i in range(N_CORES))
    return np.float32(1.0 - S / N_CELLS)


# revision 8
# speedup vs baseline: 1.2428x; 1.1596x over previous
"""IsoVelo kNN cosine-similarity loss on 8 Trainium2 NeuronCores.

Strategy: data-parallel over the 100k cells in blocks of 12544 per core
(128 partitions x 98 cells; the last core is zero-padded). Host sends
one packed uint16 blob per core, [12544, 49] (98 bytes per cell): the
neighbor indices' uint16 low halves (30), two uint16 words holding the
30 index high bits (17-bit indices), the state-table row quantized to
int8 with a global scale (17), and the velocity v = predict - state as
an int8 unit direction (17). Cosine similarity is scale-invariant in
both v and the displacement, so the int8 scales cancel and never need
decoding. On device each core carves out its int8 state rows,
AllGathers them into a full table (rank blocks of 12544 rows make
global row == cell id), unpacks indices with DVE integer ops, then
fetches neighbor rows with indirect DMA gathers (17B rows). Per-pair
math runs on DVE/ACT in fp16/fp32 (integer-valued fp16 stays exact
through the critical subtraction); per-core partial sums are reduced
with a 1-wide PE matmul and summed on the host.

Wall-clock wins over the naive approach are host-side: ~10MB shipped
per call in one array instead of ~80MB in several (no 8x-replicated
fp32 table), a jitted executable cached in-process so repeat calls skip
re-tracing and repeated remote output fetches, and the jax persistent
compilation cache so a fresh process can reuse the compiled NEFF.
"""

import os
import tempfile
from concurrent.futures import ThreadPoolExecutor

import numpy as np
import jax

_cache_dir = os.path.join(tempfile.gettempdir(), "jax_comp_cache")
try:
    jax.config.update("jax_compilation_cache_dir", _cache_dir)
    jax.config.update("jax_persistent_cache_min_compile_time_secs", 0)
    jax.config.update("jax_persistent_cache_min_entry_size_bytes", 0)
except Exception:
    pass

import concourse.bass as bass
import concourse.bacc as bacc
import concourse.mybir as mybir
from concourse.bass import AP, IndirectOffsetOnAxis
from concourse.tile import TileContext
from concourse import bass_utils

F32 = mybir.dt.float32
F16 = mybir.dt.float16
I32 = mybir.dt.int32
U16 = mybir.dt.uint16
I8 = mybir.dt.int8

N_CELLS = 100000
N_ISO = 16
D = N_ISO + 1          # 17
K = 30                 # neighbors per cell (indices[:, 1:31])
N_CORES = 8
PER_PART = 98          # cells per partition
SHARD = 128 * PER_PART          # 12544 cells per core (zero-padded globally)
PAD_TOTAL = N_CORES * SHARD     # 100352
T = 7                  # cells per partition per chunk
NCH = PER_PART // T    # 14 chunks per core
PK = T * K             # 210 pairs per partition per chunk
PY = PK * D            # 3570 gathered elements per partition per chunk

ROW = K + 2 + D        # 49 uint16 per packed row (98 bytes)
ROW_B = 2 * ROW        # 98 bytes per row
OFF_LO = 0             # u16 index low halves, elements [0, 30)
OFF_H0 = K             # u16 high bits of idx k=0..15
OFF_H1 = K + 1         # u16 high bits of idx k=16..29
OFF_TAB_B = 2 * (K + 2)  # 64: int8 state row, bytes [64, 81)
OFF_V_B = OFF_TAB_B + D  # 81: int8 velocity direction, bytes [81, 98)

_CACHED = {}


def _fv(ap, dims):
    """View a tile AP with custom free dims (list of [step, count] in
    elements), keeping its partition entry."""
    return AP(ap.tensor, ap.offset, [ap.ap[0]] + [list(d) for d in dims])


def _ov(ap, off, dims):
    return AP(ap.tensor, ap.offset + off, [ap.ap[0]] + [list(d) for d in dims])


def _build_bass():
    nc = bacc.Bacc(num_devices=N_CORES)
    blob = nc.declare_dram_parameter("blob", [SHARD, ROW], U16, isOutput=False)
    out = nc.declare_dram_parameter("out", [1, 1], F32, isOutput=True)

    tabfull = nc.dram_tensor("tabfull", [PAD_TOTAL, D], I8, addr_space="Shared")
    blob8 = blob.bitcast(I8)            # [SHARD, 98] int8 view

    with TileContext(nc) as tc:
        with (
            tc.tile_pool(name="dram", bufs=1, space="DRAM") as dp,
            tc.tile_pool(name="const", bufs=1) as cp,
            tc.tile_pool(name="io", bufs=3) as iop,
            tc.tile_pool(name="big", bufs=2) as bp,
            tc.tile_pool(name="small", bufs=2) as sp,
            tc.tile_pool(name="psum", bufs=1, space="PSUM") as pp,
        ):
            # Extract this core's int8 state rows and AllGather into the
            # full table.
            myin = dp.tile([SHARD, D], I8)
            nc.gpsimd.dma_start(
                myin[:], AP(blob8, OFF_TAB_B, [[ROW_B, SHARD], [1, D]])
            )
            nc.gpsimd.collective_compute(
                "AllGather",
                mybir.AluOpType.bypass,
                replica_groups=[list(range(N_CORES))],
                ins=[myin.opt()],
                outs=[tabfull[:].opt()],
            )

            acc = cp.tile([128, 1], F32)
            ones = cp.tile([128, 1], F32)
            nc.vector.memset(acc[:], 0.0)
            nc.vector.memset(ones[:], 1.0)

            # Resident packed shard, partition-major by construction:
            # partition p owns cells [p*98, (p+1)*98).
            blob_s = cp.tile([128, PER_PART * ROW], U16)
            nc.sync.dma_start(
                out=blob_s[:],
                in_=AP(blob[:].tensor, 0,
                       [[PER_PART * ROW, 128], [1, PER_PART * ROW]]),
            )
            bs8 = blob_s[:].bitcast(I8)

            # Upcast the int8 state and velocity rows to fp16 residents
            # (values are integers in [-127, 127]: exact in fp16).
            tab16 = cp.tile([128, PER_PART * D], F16)
            v16 = cp.tile([128, PER_PART * D], F16)
            nc.vector.tensor_copy(
                out=tab16[:],
                in_=_ov(bs8, OFF_TAB_B, [[ROW_B, PER_PART], [1, D]]))
            nc.vector.tensor_copy(
                out=v16[:],
                in_=_ov(bs8, OFF_V_B, [[ROW_B, PER_PART], [1, D]]))

            # Unpack 17-bit indices: idx = lo + (((h >> k) & 1) << 16),
            # where h's bits 0..15 come from word h0 and 16..29 from h1.
            kv = cp.tile([128, K], I32)
            nc.gpsimd.iota(kv[:], pattern=[[1, K]], base=0, channel_multiplier=0)
            h0 = cp.tile([128, PER_PART], I32)
            h1 = cp.tile([128, PER_PART], I32)
            nc.vector.tensor_copy(
                out=h0[:], in_=_ov(blob_s[:], OFF_H0, [[ROW, PER_PART]]))
            nc.vector.tensor_copy(
                out=h1[:], in_=_ov(blob_s[:], OFF_H1, [[ROW, PER_PART]]))
            sh_t = cp.tile([128, PER_PART * K], I32)
            nc.vector.tensor_tensor(
                out=_fv(sh_t[:], [[K, PER_PART], [1, 16]]),
                in0=_fv(h0[:], [[1, PER_PART], [0, 16]]),
                in1=_fv(kv[:], [[0, PER_PART], [1, 16]]),
                op=mybir.AluOpType.logical_shift_right,
            )
            nc.vector.tensor_tensor(
                out=_ov(sh_t[:], 16, [[K, PER_PART], [1, K - 16]]),
                in0=_fv(h1[:], [[1, PER_PART], [0, K - 16]]),
                in1=_fv(kv[:], [[0, PER_PART], [1, K - 16]]),
                op=mybir.AluOpType.logical_shift_right,
            )
            nc.vector.tensor_scalar(
                out=sh_t[:], in0=sh_t[:], scalar1=1, scalar2=16,
                op0=mybir.AluOpType.bitwise_and,
                op1=mybir.AluOpType.logical_shift_left,
            )
            idx_s = cp.tile([128, PER_PART * K], I32)
            nc.vector.tensor_copy(
                out=idx_s[:],
                in_=_ov(blob_s[:], OFF_LO, [[ROW, PER_PART], [1, K]]))
            nc.vector.tensor_add(out=idx_s[:], in0=idx_s[:], in1=sh_t[:])

            # |v|^2 per cell, all 98 cells at once.
            vsq = cp.tile([128, PER_PART * D], F32)
            nc.scalar.square(out=vsq[:], in_=v16[:])
            vn2 = cp.tile([128, PER_PART], F32)
            nc.vector.tensor_reduce(
                out=vn2[:], in_=_fv(vsq[:], [[D, PER_PART], [1, D]]),
                axis=mybir.AxisListType.X, op=mybir.AluOpType.add,
            )

            for ch in range(NCH):
                el_off = ch * T * D
                idxsl = idx_s[:, ch * PK:(ch + 1) * PK]

                Y = iop.tile([128, PY], I8, tag="Y")
                nc.gpsimd.indirect_dma_start(
                    out=Y[:],
                    out_offset=None,
                    in_=tabfull[:],
                    in_offset=IndirectOffsetOnAxis(ap=idxsl, axis=0),
                )
                Y16 = bp.tile([128, PY], F16, tag="Y16")
                nc.vector.tensor_copy(out=Y16[:], in_=Y[:])

                # neighbor displacement vn = Y - x (x broadcast over K);
                # both sides are integer-valued fp16, so vn is exact and a
                # duplicate neighbor (j == i) gives vn == 0 bit-exactly,
                # making cos == 0 to match the reference's denom==0 guard.
                vn = bp.tile([128, PY], F16, tag="vn")
                Y4 = _fv(Y16[:], [[K * D, T], [D, K], [1, D]])
                xb = _ov(tab16[:], el_off, [[D, T], [0, K], [1, D]])
                vn4 = _fv(vn[:], [[K * D, T], [D, K], [1, D]])
                nc.vector.tensor_tensor(
                    out=vn4, in0=Y4, in1=xb, op=mybir.AluOpType.subtract
                )

                # dots = sum_d vn * v (v broadcast over K)
                tt = bp.tile([128, PY], F16, tag="scratch")
                vb = _ov(v16[:], el_off, [[D, T], [0, K], [1, D]])
                tt4 = _fv(tt[:], [[K * D, T], [D, K], [1, D]])
                nc.vector.tensor_tensor(out=tt4, in0=vn4, in1=vb, op=mybir.AluOpType.mult)
                dots = sp.tile([128, PK], F32, tag="dots")
                nc.vector.tensor_reduce(
                    out=dots[:], in_=tt4,
                    axis=mybir.AxisListType.X, op=mybir.AluOpType.add,
                )

                # d2 = |vn|^2 (square on ACT to offload DVE)
                t2 = bp.tile([128, PY], F32, tag="sq")
                nc.scalar.square(out=t2[:], in_=vn[:])
                d2 = sp.tile([128, PK], F32, tag="d2")
                nc.vector.tensor_reduce(
                    out=d2[:], in_=_fv(t2[:], [[K * D, T], [D, K], [1, D]]),
                    axis=mybir.AxisListType.X, op=mybir.AluOpType.add,
                )

                # denom^2 = d2 * |v|^2, clamped away from zero.
                d2v = sp.tile([128, PK], F32, tag="d2v")
                vn2b = _ov(vn2[:], ch * T, [[1, T], [0, K]])
                nc.vector.tensor_tensor(
                    out=_fv(d2v[:], [[K, T], [1, K]]),
                    in0=_fv(d2[:], [[K, T], [1, K]]),
                    in1=vn2b, op=mybir.AluOpType.mult,
                )
                nc.vector.tensor_scalar_max(d2v[:], d2v[:], 1e-30)

                q = sp.tile([128, PK], F32, tag="q")
                nc.scalar.sqrt(out=q[:], in_=d2v[:])
                r = sp.tile([128, PK], F32, tag="r")
                nc.vector.reciprocal(out=r[:], in_=q[:])
                s = sp.tile([128, PK], F32, tag="s")
                nc.vector.tensor_mul(out=s[:], in0=dots[:], in1=r[:])

                # max over neighbors, then accumulate per partition
                m = sp.tile([128, T], F32, tag="m")
                nc.vector.tensor_reduce(
                    out=m[:], in_=_fv(s[:], [[K, T], [1, K]]),
                    axis=mybir.AxisListType.X, op=mybir.AluOpType.max,
                )
                msum = sp.tile([128, 1], F32, tag="msum")
                nc.vector.tensor_reduce(
                    out=msum[:], in_=m[:],
                    axis=mybir.AxisListType.X, op=mybir.AluOpType.add,
                )
                nc.vector.tensor_add(out=acc[:], in0=acc[:], in1=msum[:])

            ps = pp.tile([1, 1], F32)
            nc.tensor.matmul(out=ps[:], lhsT=acc[:], rhs=ones[:], start=True, stop=True)
            sres = cp.tile([1, 1], F32)
            nc.vector.tensor_copy(out=sres[:], in_=ps[:])
            nc.sync.dma_start(out=out[:], in_=sres[:])

    nc.compile()
    return nc


def _get_nc():
    if "nc" not in _CACHED:
        _CACHED["nc"] = _build_bass()
    return _CACHED["nc"]


class _Runner:
    """In-process cached SPMD runner: the jitted shard_map executable is
    built once, so repeat calls skip re-tracing/compile-cache lookups and
    fetch each output exactly once. Mirrors bass2jax.run_bass_via_pjrt."""

    def __init__(self, nc, n_cores):
        from concourse import bass2jax

        bass2jax.install_neuronx_cc_hook()
        assert nc.dbg_addr is None
        partition_name = (
            nc.partition_id_tensor.name if nc.partition_id_tensor else None
        )

        in_names, out_names, out_avals = [], [], []
        for alloc in nc.m.functions[0].allocations:
            if not isinstance(alloc, mybir.MemoryLocationSet):
                continue
            name = alloc.memorylocations[0].name
            if alloc.kind == "ExternalInput":
                if name != partition_name:
                    in_names.append(name)
            elif alloc.kind == "ExternalOutput":
                out_names.append(name)
                out_avals.append(jax.core.ShapedArray(
                    tuple(alloc.tensor_shape), mybir.dt.np(alloc.dtype)))
        self.in_names = list(in_names)
        self.n_cores = n_cores
        self._out_info = [(tuple(a.shape), a.dtype) for a in out_avals]
        n_in, n_out = len(in_names), len(out_names)
        all_in_names = tuple(
            in_names + out_names + ([partition_name] if partition_name else [])
        )

        def _body(*args):
            operands = list(args)
            if partition_name is not None:
                operands.append(bass2jax.partition_id_tensor())
            outs = bass2jax._bass_exec_p.bind(
                *operands,
                out_avals=tuple(out_avals),
                in_names=all_in_names,
                out_names=tuple(out_names),
                lowering_input_output_aliases=(),
                sim_require_finite=True,
                sim_require_nnan=True,
                nc=nc,
            )
            return tuple(outs)

        from jax.experimental.shard_map import shard_map
        from jax.sharding import Mesh, PartitionSpec

        devices = jax.devices()[:n_cores]
        assert len(devices) == n_cores
        mesh = Mesh(np.asarray(devices), ("core",))
        self._fn = jax.jit(
            shard_map(
                _body, mesh=mesh,
                in_specs=(PartitionSpec("core"),) * (n_in + n_out),
                out_specs=(PartitionSpec("core"),) * n_out,
                check_rep=False,
            ),
            donate_argnums=tuple(range(n_in, n_in + n_out)),
            keep_unused=True,
        )

    def __call__(self, arrays_by_name):
        ins = [arrays_by_name[n] for n in self.in_names]
        zeros = [np.zeros((self.n_cores * s[0], *s[1:]), d)
                 for s, d in self._out_info]
        outs = self._fn(*ins, *zeros)
        return [np.asarray(o) for o in outs]


def _get_runner(nc):
    if "runner" not in _CACHED:
        _CACHED["runner"] = _Runner(nc, N_CORES)
    return _CACHED["runner"]


def _get_pool():
    if "pool" not in _CACHED:
        _CACHED["pool"] = ThreadPoolExecutor(3)
    return _CACHED["pool"]


def _prepare_global(unsplice, splices, unsplice_predict, splice_predicts, indices):
    u = np.asarray(unsplice, dtype=np.float32).reshape(N_CELLS)
    s = np.asarray(splices, dtype=np.float32).reshape(N_CELLS, N_ISO)
    up = np.asarray(unsplice_predict, dtype=np.float32).reshape(N_CELLS)
    sp_ = np.asarray(splice_predicts, dtype=np.float32).reshape(N_CELLS, N_ISO)

    buf = np.zeros((PAD_TOTAL, ROW), dtype=np.uint16)
    i8 = buf.view(np.int8)              # [PAD_TOTAL, 98]

    def _mk_tab():
        tab = np.empty((N_CELLS, D), dtype=np.float32)
        tab[:, 0] = u
        tab[:, 1:] = s
        # Global int8 quantization; the scale cancels in the cosine.
        scale = 127.0 / max(float(np.abs(tab).max()), 1e-30)
        i8[:N_CELLS, OFF_TAB_B:OFF_TAB_B + D] = np.round(
            tab * scale).astype(np.int8)

    def _mk_v():
        v = np.empty((N_CELLS, D), dtype=np.float32)
        v[:, 0] = up - u
        v[:, 1:] = sp_ - s
        # Per-cell unit direction in int8; cos is scale-invariant in v.
        nrm = np.linalg.norm(v, axis=1, keepdims=True)
        np.maximum(nrm, 1e-30, out=nrm)
        i8[:N_CELLS, OFF_V_B:OFF_V_B + D] = np.round(
            v * (127.0 / nrm)).astype(np.int8)

    def _mk_idx():
        idx = np.asarray(indices).reshape(N_CELLS, K + 1)[:, 1:]
        if idx.dtype != np.int32:
            idx = idx.astype(np.int32)
        buf[:N_CELLS, OFF_LO:OFF_LO + K] = idx.astype(np.uint16)
        # bit k = (idx[:,k] >= 2^16); packbits LSB-first -> 4 bytes -> 2 u16
        buf[:N_CELLS, OFF_H0:OFF_H0 + 2] = np.packbits(
            idx >= 65536, axis=1, bitorder="little").view(np.uint16)

    futs = [_get_pool().submit(f) for f in (_mk_idx, _mk_tab, _mk_v)]
    for f in futs:
        f.result()
    return {"blob": buf}


def _split_per_core(arrays):
    in_maps = []
    for c in range(N_CORES):
        sl = slice(c * SHARD, (c + 1) * SHARD)
        in_maps.append({k: v[sl] for k, v in arrays.items()})
    return in_maps


def kernel(unsplice, splices, unsplice_predict, splice_predicts, indices,
           _trace=False):
    nc = _get_nc()
    arrays = _prepare_global(
        unsplice, splices, unsplice_predict, splice_predicts, indices
    )
    if _trace or _CACHED.get("fallback"):
        res = bass_utils.run_bass_kernel_spmd(
            nc, _split_per_core(arrays), list(range(N_CORES)), trace=_trace
        )
        S = sum(float(res.results[i]["out"][0, 0]) for i in range(N_CORES))
        loss = np.float32(1.0 - S / N_CELLS)
        if _trace:
            return loss, res
        return loss
    try:
        outs = _get_runner(nc)(arrays)
        S = float(outs[0].sum())
    except Exception:
        _CACHED["fallback"] = True
        res = bass_utils.run_bass_kernel_spmd(
            nc, _split_per_core(arrays), list(range(N_CORES))
        )
        S = sum(float(res.results[i]["out"][0, 0]) for i in range(N_CORES))
    return np.float32(1.0 - S / N_CELLS)
